# revision 13
# baseline (speedup 1.0000x reference)
"""Causal self-attention (k/q swapped variant) on 8 Trainium2 NeuronCores.

Problem (hardcoded shapes): B=2, N=2048, D=1024, H=16, DH=64.
  kqv = einsum('bnd,hde->bhne', x, Wkqv) + bkqv   ; split -> k, q, v
  A[b,h,n,m] = k[b,h,n]·q[b,h,m] / sqrt(DH), causal mask m<=n, softmax over m
  sa = A @ v ; concat heads ; out = sa @ Wo + bo

Sharding: batch x heads — core c owns batch c//4 and heads 4*(c%4)..+4 (two
head-pairs A/B), computes its partial output projection sa_local @ Wo[rows]
in bf16 over its single batch, and the host sums 4 partials per batch (+bo)
in fp32. This halves both the x input DMA and the partial-output DMA vs
all-batches-per-core head sharding.

Per-core device kernel (all matmul operands bf16, fp32 PSUM accumulation):
  - x is pre-transposed on host to xt = x[b].T ([D, N]) so the contraction
    dim d lands on SBUF partitions; one copy shared by both head-pairs.
  - scores are computed transposed, S^T[m, n] = q[m]·k[n], so softmax's
    reduction dim m sits on partitions; both heads of a pair live in ONE
    [128, 1024] PSUM tile (2 banks) so off-diagonal chunks need a single wide
    exp() on the ACT engine. The denominator comes free from the PV matmul by
    augmenting v with 64 ones columns.
  - the chunk loop is software-pipelined: scores+exp of chunk ci+1 are
    emitted before the PV of chunk ci; projection / output-projection work is
    woven between chunks in ~0.9us slots to fill the remaining PE gaps.
  - output projection accumulates both pairs' contributions on device; the
    two psum halves of a block alternate over two banks so the next block's
    matmuls overlap the previous block's psum->bf16 cast.
  - PSUM budget (8 banks): scores 2x[128,1024] (4) + PV accumulators 2 +
    proj/outproj bank 1 + transpose/outproj bank 1.
"""

import numpy as np
import ml_dtypes

B = 2
N = 2048
D = 1024
H = 16
DH = 64
NCORES = 8
HL = 2                    # heads per pair
NPAIR = 2                 # head-pairs per core
DC = D // 128             # contraction chunks = 8
NB = N // 128             # 128-row blocks = 16
NJ = N // 512             # 512-col blocks = 4

BF16 = ml_dtypes.bfloat16

_CACHE = {}


def _build():
    import concourse.bass as bass
    import concourse.mybir as mybir
    import concourse.tile as tile
    from concourse import bacc
    from contextlib import ExitStack

    f32 = mybir.dt.float32
    bf16 = mybir.dt.bfloat16
    Exp = mybir.ActivationFunctionType.Exp

    nc = bacc.Bacc("TRN2", target_bir_lowering=False, debug=False,
                   enable_asserts=False, num_devices=NCORES)

    xt_d = nc.dram_tensor("xt", [D, N], bf16, kind="ExternalInput")
    # k/q/v weights arrive pre-shuffled to the SBUF layout [128, DC*128]
    # (partition = within-chunk row, free = (chunk, head-col)), per pair
    wk_d = {p: nc.dram_tensor(f"wk{p}", [128, DC * 128], bf16,
                              kind="ExternalInput") for p in range(NPAIR)}
    wq_d = {p: nc.dram_tensor(f"wq{p}", [128, DC * 128], bf16,
                              kind="ExternalInput") for p in range(NPAIR)}
    wv_d = {p: nc.dram_tensor(f"wv{p}", [128, DC * 128], bf16,
                              kind="ExternalInput") for p in range(NPAIR)}
    wo_d = {p: nc.dram_tensor(f"wo{p}", [128, D], bf16,
                              kind="ExternalInput") for p in range(NPAIR)}
    bk_d = {p: nc.dram_tensor(f"bk{p}", [128, 1], f32,
                              kind="ExternalInput") for p in range(NPAIR)}
    bq_d = {p: nc.dram_tensor(f"bq{p}", [128, 1], f32,
                              kind="ExternalInput") for p in range(NPAIR)}
    bv_d = {p: nc.dram_tensor(f"bv{p}", [128, 1], f32,
                              kind="ExternalInput") for p in range(NPAIR)}
    eye_d = nc.dram_tensor("eye2", [128, 128], bf16, kind="ExternalInput")
    m01_d = nc.dram_tensor("m01", [128, 128], bf16, kind="ExternalInput")
    out_d = nc.dram_tensor("out", [N, D], bf16, kind="ExternalOutput")

    with tile.TileContext(nc) as tc, ExitStack() as ctx:
        const = ctx.enter_context(tc.tile_pool(name="const", bufs=1))
        xt_pool = ctx.enter_context(tc.tile_pool(name="xt", bufs=1))
        kq_pool = ctx.enter_context(tc.tile_pool(name="kq", bufs=6))
        v_pool = ctx.enter_context(tc.tile_pool(name="v", bufs=2))
        sa_pool = ctx.enter_context(tc.tile_pool(name="sa", bufs=2))
        pt_pool = ctx.enter_context(tc.tile_pool(name="pt", bufs=6))
        rc_pool = ctx.enter_context(tc.tile_pool(name="rc", bufs=2))
        ob_pool = ctx.enter_context(tc.tile_pool(name="ob", bufs=4))
        s_ps = ctx.enter_context(tc.tile_pool(name="s_ps", bufs=2, space="PSUM"))
        pv_ps = ctx.enter_context(tc.tile_pool(name="pv_ps", bufs=2, space="PSUM"))
        wv_ps = ctx.enter_context(tc.tile_pool(name="wv_ps", bufs=1, space="PSUM"))
        tp_ps = ctx.enter_context(tc.tile_pool(name="tp_ps", bufs=1, space="PSUM"))

        # ---- DMA issue order: pair-A weights -> xt half-0 pieces (paced,
        # two rings) -> small consts -> xt half-1 pieces -> pair-B weights.
        wk_sb, wq_sb, wv_sb, wo_sb = {}, {}, {}, {}
        bk_sb, bq_sb, bv_sb = {}, {}, {}
        for p in range(NPAIR):
            wk_sb[p] = const.tile([128, DC * 128], bf16, name=f"wk_sb{p}")
            wq_sb[p] = const.tile([128, DC * 128], bf16, name=f"wq_sb{p}")
            wv_sb[p] = const.tile([128, DC * 128], bf16, name=f"wv_sb{p}")
            wo_sb[p] = const.tile([128, D], bf16, name=f"wo_sb{p}")
            bk_sb[p] = const.tile([128, 1], f32, name=f"bk_sb{p}")
            bq_sb[p] = const.tile([128, 1], f32, name=f"bq_sb{p}")
            bv_sb[p] = const.tile([128, 1], f32, name=f"bv_sb{p}")
        eye_sb = const.tile([128, 128], bf16, name="eye_sb")
        m01_sb = const.tile([128, 128], bf16, name="m01_sb")

        v_sb = {}
        sa_sb = {}
        for p in range(NPAIR):
            v_sb[p] = v_pool.tile([128, NB * 192], bf16, name=f"v_p{p}",
                                  tag="v")
            sa_sb[p] = sa_pool.tile([128, N], bf16, name=f"sa_p{p}", tag="sa")
            nc.gpsimd.memset(
                v_sb[p][:].rearrange("p (nb g) -> p nb g", g=192)[:, :, 64:128],
                1.0)

        # v_sb "ones" memsets go first on the gpsimd queue so its xt DMA
        # issues (below) don't delay them; the scalar queue carries no DMAs
        # at all so the first exp() can issue as soon as scores land.
        nc.sync.dma_start(wk_sb[0][:], wk_d[0].ap())
        nc.sync.dma_start(wq_sb[0][:], wq_d[0].ap())
        nc.sync.dma_start(wv_sb[0][:], wv_d[0].ap())
        xt0 = {}   # (dc, half) -> [128, 1024]
        for half in range(2):
            for dc in range(DC):
                t = xt_pool.tile([128, 1024], bf16, name=f"xt_{dc}_{half}",
                                 tag="xt", bufs=2 * DC)
                eng = nc.sync if dc % 2 == 0 else nc.gpsimd
                eng.dma_start(t[:], xt_d.ap()[dc * 128:(dc + 1) * 128,
                                              half * 1024:(half + 1) * 1024])
                xt0[dc, half] = t
            if half == 0:
                nc.sync.dma_start(eye_sb[:], eye_d.ap())
                nc.sync.dma_start(m01_sb[:], m01_d.ap())
                for p in range(NPAIR):
                    nc.sync.dma_start(bk_sb[p][:], bk_d[p].ap())
                    nc.sync.dma_start(bq_sb[p][:], bq_d[p].ap())
                    nc.sync.dma_start(bv_sb[p][:], bv_d[p].ap())
        nc.sync.dma_start(wk_sb[1][:], wk_d[1].ap())
        nc.sync.dma_start(wq_sb[1][:], wq_d[1].ap())
        nc.sync.dma_start(wv_sb[1][:], wv_d[1].ap())
        nc.sync.dma_start(wo_sb[0][:], wo_d[0].ap())
        nc.sync.dma_start(wo_sb[1][:], wo_d[1].ap())

        def xt_ap(dc, c0, c1):
            half = c0 // 1024
            return xt0[dc, half][:, c0 - half * 1024:c1 - half * 1024]

        # ---- per-pair tensors
        k2 = {p: kq_pool.tile([128, N], bf16, name=f"k2_p{p}", tag="kq")
              for p in range(NPAIR)}
        q2 = {p: kq_pool.tile([128, N], bf16, name=f"q2_p{p}", tag="kq")
              for p in range(NPAIR)}
        vt = {p: kq_pool.tile([128, N], bf16, name=f"vt_p{p}", tag="kq")
              for p in range(NPAIR)}
        groups = {p: ((wk_sb[p], bk_sb[p], k2[p]), (wq_sb[p], bq_sb[p], q2[p]),
                      (wv_sb[p], bv_sb[p], vt[p])) for p in range(NPAIR)}

        def make_proj_slots(p, gi, nj):
            """One [128, 512] projection group as two weave slots (dc 0-3 and
            dc 4-7 + bias add), sharing the wv PSUM bank."""
            st = {}

            def part(d0, d1):
                def go():
                    if d0 == 0:
                        st["ps"] = wv_ps.tile([128, 512], f32, name="wvps",
                                              tag="wv")
                    ps = st["ps"]
                    w_sb, bias_sb, dst = groups[p][gi]
                    for dc in range(d0, d1):
                        nc.tensor.matmul(
                            ps[:], w_sb[:, dc * 128:(dc + 1) * 128],
                            xt_ap(dc, nj * 512, (nj + 1) * 512),
                            start=(dc == 0), stop=(dc == DC - 1))
                    if d1 == DC:
                        nc.vector.tensor_scalar_add(
                            dst[:, nj * 512:(nj + 1) * 512], ps[:], bias_sb[:])
                return go
            return [part(0, 4), part(4, DC)]

        def make_op_slot(nb, cast_eng=None):
            """Output projection of one 128-row block: both pairs accumulate;
            the two column halves alternate over the wv / tp banks so casts
            overlap the next matmuls; one [128, 1024] bf16 DMA per block."""
            def go():
                ob = ob_pool.tile([128, 1024], bf16, name="ob", tag="ob")
                for half in range(2):
                    pool, tag = (wv_ps, "wv") if half == 0 else (tp_ps, "tp")
                    op = pool.tile([128, 512], f32, name="opps", tag=tag,
                                   padded_shape=[128, 512])
                    for p in range(NPAIR):
                        nc.tensor.matmul(
                            op[:], sa_sb[p][:, nb * 128:(nb + 1) * 128],
                            wo_sb[p][:, half * 512:(half + 1) * 512],
                            start=(p == 0), stop=(p == NPAIR - 1))
                    eng = cast_eng or nc.vector
                    if eng is nc.scalar:
                        eng.copy(ob[:, half * 512:(half + 1) * 512], op[:])
                    else:
                        eng.tensor_copy(ob[:, half * 512:(half + 1) * 512],
                                        op[:])
                nc.sync.dma_start(out_d.ap()[nb * 128:(nb + 1) * 128, :],
                                  ob[:])
            return go

        def kqv0_start():
            """Pair-A k/q/v projections for nj=0, d-chunk-major so the PE is
            paced by the half-0 xt piece DMAs (k/q in one score-pool tile)."""
            kq0s = s_ps.tile([128, 1024], f32, name="kq0s", tag="s")
            v0ps = wv_ps.tile([128, 512], f32, name="v0ps", tag="wv")
            for dc in range(DC):
                nc.tensor.matmul(kq0s[:, 0:512],
                                 wk_sb[0][:, dc * 128:(dc + 1) * 128],
                                 xt_ap(dc, 0, 512),
                                 start=(dc == 0), stop=(dc == DC - 1))
                nc.tensor.matmul(kq0s[:, 512:1024],
                                 wq_sb[0][:, dc * 128:(dc + 1) * 128],
                                 xt_ap(dc, 0, 512),
                                 start=(dc == 0), stop=(dc == DC - 1))
                nc.tensor.matmul(v0ps[:],
                                 wv_sb[0][:, dc * 128:(dc + 1) * 128],
                                 xt_ap(dc, 0, 512),
                                 start=(dc == 0), stop=(dc == DC - 1))
            nc.vector.tensor_scalar_add(k2[0][:, 0:512], kq0s[:, 0:512],
                                        bk_sb[0][:])
            nc.vector.tensor_scalar_add(q2[0][:, 0:512], kq0s[:, 512:1024],
                                        bq_sb[0][:])
            nc.vector.tensor_scalar_add(vt[0][:, 0:512], v0ps[:], bv_sb[0][:])

        def transpose_v(p, nb):
            """Rotate vT[dh, n] -> v[n, dh] for one 128-row chunk, both heads
            in one PE transpose."""
            tp = tp_ps.tile([128, 128], bf16, name="tp", tag="tp")
            nc.tensor.transpose(
                tp[:], vt[p][:, nb * 128:(nb + 1) * 128], eye_sb[:])
            nc.vector.tensor_copy(
                v_sb[p][:, nb * 192:nb * 192 + 64], tp[:, 0:64])
            nc.vector.tensor_copy(
                v_sb[p][:, nb * 192 + 128:nb * 192 + 192], tp[:, 64:128])

        def emit_scores(p, j, ci, state):
            """Scores (both heads into one 2-bank psum tile) + exp for one
            128-m chunk."""
            t = ci - 4 * j
            lo = 128 * t if t >= 0 else 0
            sp = s_ps.tile([128, 1024], f32, name="s", tag="s")
            pt = pt_pool.tile([128, 1024], bf16, name="pt", tag="pt")
            for h in range(HL):
                hp = 64 * h
                nc.tensor.matmul(
                    sp[:, 512 * h + lo:512 * h + 512],
                    q2[p][hp:hp + 64, ci * 128:(ci + 1) * 128],
                    k2[p][hp:hp + 64, j * 512 + lo:(j + 1) * 512],
                    start=True, stop=True)
            if t < 0:
                nc.scalar.activation(pt[:], sp[:], Exp, scale=0.125)
            else:
                # both heads' [lo:512] regions as one strided-AP instruction
                sp2 = sp[:].rearrange("p (h c) -> p h c", h=2)[:, :, lo:512]
                pt2 = pt[:].rearrange("p (h c) -> p h c", h=2)[:, :, lo:512]
                nc.scalar.activation(pt2, sp2, Exp, scale=0.125)
                for h in range(HL):
                    nc.vector.tensor_tensor(
                        pt[:, 512 * h + lo:512 * h + lo + 128],
                        pt[:, 512 * h + lo:512 * h + lo + 128],
                        m01_sb[:], mybir.AluOpType.mult)
            state[ci] = (pt, lo)

        def emit_pv(p, j, ci, pv, state, nch):
            pt, lo = state.pop(ci)
            for h in range(HL):
                nc.tensor.matmul(
                    pv[h][:, lo:512],
                    v_sb[p][:, ci * 192 + 64 * h:ci * 192 + 64 * h + 128],
                    pt[:, 512 * h + lo:512 * h + 512],
                    start=(ci == 0), stop=(ci == nch - 1))

        def att_norm(p, j, pv):
            for h in range(HL):
                # h0: psum rows 0:64 = sa, 64:128 = denom ; h1: swapped
                sa_rows = pv[h][64 * h:64 * h + 64, :]
                den_rows = pv[h][64 - 64 * h:128 - 64 * h, :]
                # denominators are sums of exp() in [~2e-3, ~3e3]: inside
                # approx_fast's domain; 18-bit accuracy is far below the bf16
                # noise of the P*V numerator. (approx_fast misreads PSUM
                # operands on HW - bounce through SBUF first.)
                den_sb = rc_pool.tile([64, 512], f32, name="den", tag="den")
                nc.vector.tensor_copy(den_sb[:], den_rows)
                rc = rc_pool.tile([64, 512], f32, name="rc", tag="rc")
                nc.vector.reciprocal_approx_fast(rc[:], den_sb[:])
                nc.vector.tensor_tensor(
                    sa_sb[p][64 * h:64 * h + 64, j * 512:(j + 1) * 512],
                    sa_rows, rc[:], mybir.AluOpType.mult)

        def att_j(p, j, weave=(), tp_delay=0):
            """One attention j-block, software-pipelined: scores/exp of chunk
            ci+1 are emitted before PV of chunk ci; weave slots fill PE gaps;
            v-rotations for the block's own rows are spread over early
            chunks."""
            pv = [pv_ps.tile([128, 512], f32, name=f"pv{h}", tag="pv")
                  for h in range(HL)]
            weave = list(weave)
            nch = 4 * (j + 1)
            state = {}
            emitted = 0
            emit_scores(p, j, 0, state)
            for ci in range(nch):
                if ci + 1 < nch:
                    emit_scores(p, j, ci + 1, state)
                target = len(weave) * (ci + 1) // nch
                while emitted < target:
                    weave[emitted]()
                    emitted += 1
                if tp_delay <= ci < tp_delay + 4:
                    transpose_v(p, 4 * j + (ci - tp_delay))
                emit_pv(p, j, ci, pv, state, nch)
            att_norm(p, j, pv)

        # ================= emission schedule =================
        def ksl(p, nj):
            return make_proj_slots(p, 0, nj)

        def qsl(p, nj):
            return make_proj_slots(p, 1, nj)

        def vsl(p, nj):
            return make_proj_slots(p, 2, nj)

        def ops(nbs):
            return [make_op_slot(nb) for nb in nbs]

        kqv0_start()
        att_j(0, 0, weave=ksl(0, 1) + qsl(0, 1))
        att_j(0, 1, weave=vsl(0, 1) + ksl(0, 2) + qsl(0, 2) + vsl(0, 2),
              tp_delay=2)
        att_j(0, 2, weave=ksl(0, 3) + qsl(0, 3) + vsl(0, 3)
              + ksl(1, 0) + qsl(1, 0) + vsl(1, 0))
        att_j(0, 3, weave=ksl(1, 1) + qsl(1, 1) + vsl(1, 1)
              + ksl(1, 2) + qsl(1, 2) + vsl(1, 2))
        att_j(1, 0, weave=ksl(1, 3) + qsl(1, 3))
        att_j(1, 1, weave=vsl(1, 3) + ops(range(0, 4)))
        att_j(1, 2, weave=ops(range(4, 8)))
        att_j(1, 3, weave=ops(range(8, 12)))
        for nb in range(12, 16):
            make_op_slot(nb, cast_eng=nc.scalar if nb % 2 == 0 else None)()

    nc.compile()
    return nc


def _get_nc():
    if "nc" not in _CACHE:
        _CACHE["nc"] = _build()
    return _CACHE["nc"]


def _prep_inputs(x, Wkqv, bkqv, Wo, bo):
    """Host-side shard prep: one input map per core (core c: batch c//4,
    head-pairs (4*(c%4), 4*(c%4)+1) and (+2, +3))."""
    tri = np.triu(np.ones((128, 128), np.float32)).astype(BF16)  # m' <= n''
    eye2 = np.eye(128, dtype=np.float32).astype(BF16)
    xts = [np.ascontiguousarray(x[b].T).astype(BF16) for b in range(B)]

    def shuf(w):
        # [D, 128] -> [128, DC*128]: partition = within-chunk row
        return np.ascontiguousarray(
            w.reshape(DC, 128, 128).transpose(1, 0, 2).reshape(128, DC * 128))

    in_maps = []
    for c in range(NCORES):
        b, m = c // 4, c % 4
        im = {"xt": xts[b], "eye2": eye2, "m01": tri}
        for p in range(NPAIR):
            h0 = 4 * m + 2 * p
            h1 = h0 + 1
            im[f"wk{p}"] = shuf(np.concatenate(
                [Wkqv[h0, :, 0:64], Wkqv[h1, :, 0:64]], axis=1)).astype(BF16)
            im[f"wq{p}"] = shuf(np.concatenate(
                [Wkqv[h0, :, 64:128], Wkqv[h1, :, 64:128]], axis=1)).astype(BF16)
            im[f"wv{p}"] = shuf(np.concatenate(
                [Wkqv[h0, :, 128:192], Wkqv[h1, :, 128:192]], axis=1)).astype(BF16)
            im[f"wo{p}"] = Wo[64 * h0:64 * h0 + 128, :].astype(BF16)
            im[f"bk{p}"] = np.ascontiguousarray(np.concatenate(
                [bkqv[h0, 0:64], bkqv[h1, 0:64]])[:, None], np.float32)
            im[f"bq{p}"] = np.ascontiguousarray(np.concatenate(
                [bkqv[h0, 64:128], bkqv[h1, 64:128]])[:, None], np.float32)
            im[f"bv{p}"] = np.ascontiguousarray(np.concatenate(
                [bkqv[h0, 128:192], bkqv[h1, 128:192]])[:, None], np.float32)
        in_maps.append(im)
    return in_maps


def kernel(x, Wkqv, bkqv, Wo, bo):
    from concourse import bass_utils

    nc = _get_nc()
    in_maps = _prep_inputs(np.asarray(x), np.asarray(Wkqv), np.asarray(bkqv),
                           np.asarray(Wo), np.asarray(bo))
    res = bass_utils.run_bass_kernel_spmd(nc, in_maps, core_ids=list(range(NCORES)))
    acc = np.zeros((B, N, D), np.float32)
    for c in range(NCORES):
        acc[c // 4] += np.asarray(res.results[c]["out"], dtype=np.float32)
    acc += np.asarray(bo)[None, None, :]
    return acc
